# revision 38
# baseline (speedup 1.0000x reference)
"""MoE top-1 routing kernel for 8 Trainium2 NeuronCores.

Strategy (expert parallelism, per the sharding hint):
  - The host computes the (tiny) router in fp32, groups tokens by their
    argmax expert, and pads each group to a common capacity C — this is
    the "token dispatch" step of expert parallelism, done at sharding
    time on the host.
  - Core e receives its expert's weights (W1[e], b1[e], W2[e], b2[e]),
    the replicated router weights, and its C dispatched tokens in
    transposed layout xT [D, C].
  - On device, core e computes the 2-layer MLP for its tokens and the
    router softmax probabilities for the same tokens.  Matmul inputs are
    bf16 (1 PE cycle/row at any free dim); PSUM accumulation, biases,
    softmax, and all outputs are fp32.
  - The host scatters y / probs back to original token order ("combine").

Everything is hardcoded for the problem shapes:
  E=8 experts, D=1024, H=2048, O=1024, 8 cores.
"""

import numpy as np
from contextlib import ExitStack

import concourse.tile as tile
from concourse import bacc, mybir
from concourse.bass import ds, ts
from concourse.bass_utils import run_bass_kernel_spmd

E = 8
D = 1024
H = 2048
O = 1024
NCORES = 8
P = 128
CGRAN = 128    # capacity granularity (bf16 matmuls run 1 cyc/row at any width)

f32 = mybir.dt.float32
f32r = mybir.dt.float32r
bf16 = mybir.dt.bfloat16

# Matmul input dtype for weights AND activations.  bf16 halves all the
# input DMA, runs 1 PE cycle/row at any free dim (so capacity granularity
# is 128 instead of 256), and gets fast weight load on real HW, at a
# ~4e-3 relative-error cost.  PSUM accumulation, biases, softmax and all
# outputs stay fp32.
W_DTYPE = "bf16"

# test.py introspection hooks
TRACE = False
LAST = {}

X_BUFS = 2
H_BUFS = 1
PSUM_BUFS = 4
Y_BUFS = 3


def _slices(C):
    """Split C into near-equal token slices of at most 512 (one fp32 PSUM
    bank), multiples of 128.  Equal widths avoid a thin tail slice whose
    short matmuls would be weight-load-bound on hardware."""
    nb = C // P
    n_sl = -(-nb // 4)          # ceil: max 4 blocks (512 tokens) per slice
    sizes = [(nb // n_sl + (i < nb % n_sl)) * P for i in range(n_sl)]
    out = []
    c = 0
    for s in sizes:
        out.append((c, s))
        c += s
    return out


def _build_program(C: int):
    """Bass program for one core: 2-layer expert MLP + router softmax
    over C tokens (transposed activations)."""
    assert C % CGRAN == 0
    slices = _slices(C)
    WMAX = 512

    nc = bacc.Bacc("TRN2", target_bir_lowering=False, debug=False,
                   num_devices=NCORES)

    wdt = bf16 if W_DTYPE == "bf16" else f32
    adt = bf16 if W_DTYPE == "bf16" else f32r   # activation matmul dtype
    xT = nc.dram_tensor("xT", [D, C], wdt, kind="ExternalInput").ap()
    w1 = nc.dram_tensor("w1", [D, H], wdt, kind="ExternalInput").ap()
    w2 = nc.dram_tensor("w2", [H, O], wdt, kind="ExternalInput").ap()
    # bias: cols 0:16 = b1 striped, 16:24 = b2 striped, 24:32 = br bcast
    biases = nc.dram_tensor("biases", [P, 32], f32, kind="ExternalInput").ap()
    wr = nc.dram_tensor("wr", [D, E], wdt, kind="ExternalInput").ap()
    yT = nc.dram_tensor("yT", [O, C], f32, kind="ExternalOutput").ap()
    probs = nc.dram_tensor("probs", [C, E], f32, kind="ExternalOutput").ap()

    K1 = D // P   # 8  contraction tiles for layer 1 / router
    K2 = H // P   # 16 contraction tiles for layer 2
    MH = H // P   # 16 output row-tiles for layer 1
    MO = O // P   # 8  output row-tiles for layer 2

    x_view = xT.rearrange("(k p) c -> p k c", p=P)
    if W_DTYPE != "bf16":
        x_view = x_view.bitcast(f32r)
    wsb_dt = bf16 if W_DTYPE == "bf16" else f32r
    w1_view = w1.rearrange("(k p) m -> p k m", p=P)
    w2_view = w2.rearrange("(k p) m -> p k m", p=P)
    if W_DTYPE != "bf16":
        w1_view = w1_view.bitcast(f32r)
        w2_view = w2_view.bitcast(f32r)
    wr_view = wr.rearrange("(k p) e -> p k e", p=P)
    if W_DTYPE != "bf16":
        wr_view = wr_view.bitcast(f32r)
    y_view = yT.rearrange("(m p) c -> p m c", p=P)

    with tile.TileContext(nc) as tc, ExitStack() as ctx:
        const = ctx.enter_context(tc.tile_pool(name="const", bufs=1))
        wpool = ctx.enter_context(tc.tile_pool(name="wpool", bufs=1))
        xpool = ctx.enter_context(tc.tile_pool(name="xpool", bufs=X_BUFS))
        hpool = ctx.enter_context(tc.tile_pool(name="hpool", bufs=H_BUFS))
        ypool = ctx.enter_context(tc.tile_pool(name="ypool", bufs=Y_BUFS))
        rpool = ctx.enter_context(tc.tile_pool(name="rpool", bufs=2))
        psum_mm = ctx.enter_context(tc.tile_pool(name="psum_mm", bufs=PSUM_BUFS, space="PSUM"))
        psum_r = ctx.enter_context(tc.tile_pool(name="psum_r", bufs=2, space="PSUM"))

        # First x slice before the bulk weight DMAs: it's the critical
        # path to the first matmul.  Chunked so the first router/L1
        # matmuls (k ascending) can start as soon as early chunks land;
        # wr (tiny, needed by the first router matmul) rides second.
        s0, w0 = slices[0]
        x0_sb = xpool.tile([P, K1, WMAX], adt, name="x_sb")
        nc.sync.dma_start(x0_sb[:, ds(0, 4), :w0],
                          x_view[:, ds(0, 4), ds(s0, w0)])
        wr_sb = const.tile([P, K1, E], adt)
        nc.sync.dma_start(wr_sb[:], wr_view)
        nc.sync.dma_start(x0_sb[:, ds(4, 4), :w0],
                          x_view[:, ds(4, 4), ds(s0, w0)])

        # Resident weights, chunked so arrival keeps pace with the L1
        # m-tile consumption order (column-block-major).  The bias DMA
        # (needed by the first L1 eviction) rides after the first chunk.
        w1_sb = wpool.tile([P, K1, H], wsb_dt)
        bias_sb = const.tile([P, 32], f32)
        b1_sb = bias_sb[:, 0:MH]
        b2_sb = bias_sb[:, MH:MH + MO]
        br_sb = bias_sb[:, MH + MO:MH + MO + E]
        for cb in range(0, H, 256):
            nc.sync.dma_start(w1_sb[:, :, ds(cb, 256)],
                              w1_view[:, :, ds(cb, 256)])
            if cb == 0:
                nc.sync.dma_start(bias_sb[:], biases)
        w2_sb = wpool.tile([P, K2, O], wsb_dt)
        for kc in range(0, K2, 8):
            for cb in range(0, O, WMAX):
                nc.sync.dma_start(w2_sb[:, ds(kc, 8), ds(cb, WMAX)],
                                  w2_view[:, ds(kc, 8), ds(cb, WMAX)])

        for si, (c0, w) in enumerate(slices):
            if si == 0:
                x_sb = x0_sb
            else:
                x_sb = xpool.tile([P, K1, WMAX], adt, name="x_sb")
                for kc in range(0, K1, 4):
                    nc.sync.dma_start(x_sb[:, ds(kc, 4), :w],
                                      x_view[:, ds(kc, 4), ds(c0, w)])

            # Router: logits -> softmax for this slice's tokens.
            pr = rpool.tile([P, WMAX // P, E], f32, name="pr")
            for t in range(w // P):
                ps = psum_r.tile([P, E], f32)
                for k in range(K1):
                    nc.tensor.matmul(
                        ps,
                        lhsT=x_sb[:, k, ts(t, P)],
                        rhs=wr_sb[:, k, :],
                        start=(k == 0),
                        stop=(k == K1 - 1),
                    )
                lg = rpool.tile([P, E], f32)
                nc.vector.tensor_add(lg[:], ps, br_sb[:])
                nmx = rpool.tile([P, 1], f32)
                nc.vector.tensor_reduce(
                    nmx[:], lg[:], mybir.AxisListType.X, mybir.AluOpType.max,
                    negate=True,
                )
                ex = rpool.tile([P, E], f32)
                sm = rpool.tile([P, 1], f32)
                nc.scalar.activation(
                    ex[:], lg[:], mybir.ActivationFunctionType.Exp,
                    bias=nmx[:], accum_out=sm[:],
                )
                rs = rpool.tile([P, 1], f32)
                nc.vector.reciprocal(rs[:], sm[:])
                nc.vector.tensor_mul(pr[:, t, :], ex[:], rs.to_broadcast((P, E)))
            nc.sync.dma_start(
                probs.rearrange("(t p) e -> p t e", p=P)[:, ds(c0 // P, w // P), :],
                pr[:, :w // P, :],
            )

            # Layer 1: hT[mh, :] = relu(W1[:, mh].T @ x + b1[mh])
            h_sb = hpool.tile([P, K2, WMAX], adt)
            for mh in range(MH):
                ps = psum_mm.tile([P, WMAX], f32)
                for k in range(K1):
                    nc.tensor.matmul(
                        ps[:, :w],
                        lhsT=w1_sb[:, k, ts(mh, P)],
                        rhs=x_sb[:, k, :w],
                        start=(k == 0),
                        stop=(k == K1 - 1),
                    )
                nc.scalar.activation(
                    h_sb[:, mh, :w], ps[:, :w],
                    mybir.ActivationFunctionType.Relu,
                    bias=b1_sb[:, ds(mh, 1)],
                )

            # Layer 2: yT[mo, :] = W2[:, mo].T @ h + b2[mo]
            # y staged in mo-pairs so each slice needs only 4 output DMAs.
            for mp in range(MO // 2):
                y_sb = ypool.tile([P, 2, WMAX], f32)
                for half in range(2):
                    mo = mp * 2 + half
                    ps = psum_mm.tile([P, WMAX], f32)
                    for k in range(K2):
                        nc.tensor.matmul(
                            ps[:, :w],
                            lhsT=w2_sb[:, k, ts(mo, P)],
                            rhs=h_sb[:, k, :w],
                            start=(k == 0),
                            stop=(k == K2 - 1),
                        )
                    nc.vector.tensor_add(
                        y_sb[:, half, :w], ps[:, :w],
                        b2_sb[:, ds(mo, 1)].to_broadcast((P, w)),
                    )
                nc.sync.dma_start(y_view[:, ds(mp * 2, 2), ds(c0, w)],
                                  y_sb[:, :, :w])

    nc.compile()
    return nc


def _w_cast(w):
    if W_DTYPE == "bf16":
        import ml_dtypes
        return np.ascontiguousarray(w.astype(ml_dtypes.bfloat16))
    return np.ascontiguousarray(w)


def _prepare(x, Wr, br, W1, b1, W2, b2):
    """Host-side dispatch: route tokens, group by expert, build per-core
    input maps.  Returns (C, in_maps, idx_per_core, counts)."""
    x = np.ascontiguousarray(np.asarray(x, dtype=np.float32))
    Wr = np.ascontiguousarray(np.asarray(Wr, dtype=np.float32))
    br = np.asarray(br, dtype=np.float32)
    W1 = np.asarray(W1, dtype=np.float32)
    b1 = np.asarray(b1, dtype=np.float32)
    W2 = np.asarray(W2, dtype=np.float32)
    b2 = np.asarray(b2, dtype=np.float32)

    # Host-side router (dispatch decision): fp32, same formulation as the
    # reference (softmax then argmax; softmax is monotonic).
    logits = x @ Wr + br
    mx = logits.max(axis=-1, keepdims=True)
    ex = np.exp(logits - mx)
    probs_host = ex / ex.sum(axis=-1, keepdims=True)
    routes = probs_host.argmax(axis=-1)
    counts = np.bincount(routes, minlength=E).astype(np.float32)

    order = np.argsort(routes, kind="stable")
    bounds = np.zeros(E + 1, dtype=np.int64)
    np.cumsum(counts.astype(np.int64), out=bounds[1:])

    C = int(max(int(counts.max()), CGRAN))
    C = ((C + CGRAN - 1) // CGRAN) * CGRAN

    in_maps = []
    idx_per_core = []
    for e in range(E):
        idx = order[bounds[e]:bounds[e + 1]]
        idx_per_core.append(idx)
        if W_DTYPE == "bf16":
            import ml_dtypes
            xdt = ml_dtypes.bfloat16
        else:
            xdt = np.float32
        xTe = np.zeros((D, C), dtype=xdt)
        xTe[:, :len(idx)] = x[idx].T.astype(xdt)
        biases = np.empty((P, 32), dtype=np.float32)
        biases[:, 0:16] = b1[e].reshape(H // P, P).T
        biases[:, 16:24] = b2[e].reshape(O // P, P).T
        biases[:, 24:32] = br
        in_maps.append({
            "xT": xTe,
            "w1": _w_cast(W1[e]),
            "w2": _w_cast(W2[e]),
            "biases": biases,
            "wr": _w_cast(Wr),
        })
    return C, in_maps, idx_per_core, counts


def kernel(x, Wr, br, W1, b1, W2, b2):
    N = np.asarray(x).shape[0]
    C, in_maps, idx_per_core, counts = _prepare(x, Wr, br, W1, b1, W2, b2)
    nc = _build_program(C)

    res = run_bass_kernel_spmd(
        nc, in_maps, list(range(NCORES)),
        trace=TRACE,
        **({"trace_cores": [0]} if TRACE else {}),
    )
    LAST["exec_time_ns"] = res.exec_time_ns
    LAST["results"] = res.results

    out = np.empty((N, O), dtype=np.float32)
    probs = np.empty((N, E), dtype=np.float32)
    for e in range(E):
        idx = idx_per_core[e]
        if len(idx) == 0:
            continue
        yTe = res.results[e]["yT"]
        out[idx] = yTe[:, :len(idx)].T
        probs[idx] = res.results[e]["probs"][:len(idx)]

    return out, probs, counts
